# revision 6
# baseline (speedup 1.0000x reference)
"""Trainium2 Bass kernel for nn_Aggregation_Module (GNN per-node-type GRU update).

Computation (per node n, with t = node_type[n]):
    x  = relu(agg_msg[n])                  # [256]
    y  = relu(x @ W_att[t] + b_att[t])     # [64]
    gi = y @ W_in[t] + b_in[t]             # [192]
    gh = h[n] @ W_hid[t] + b_hid[t]        # [192]
    r  = sigmoid(gi_r + gh_r); z = sigmoid(gi_z + gh_z)
    n_ = tanh(gi_n + r * gh_n)
    h_new = (1 - z) * n_ + z * h[n]

Sharding: type-parallel across the 8 cores (T == 8).  Core t processes all
nodes of type t (counts ~1024 each), so each core needs only its own type's
small weight set (no weight replication).  The node rows are gathered on
device with indirect DMA using a per-core row-index list (built on host from
node_type), and results are scattered back to their original rows on device.
Row N (8192) of the padded inputs/outputs is a scratch row used by the
padding lanes.
"""

import sys

sys.path.insert(0, "/opt/trn_rl_repo")

from contextlib import ExitStack

import numpy as np

import concourse.bacc as bacc
import concourse.mybir as mybir
import concourse.tile as tile
from concourse import bass
from concourse.bass import IndirectOffsetOnAxis
from concourse.bass_utils import run_bass_kernel_spmd
from concourse.masks import make_identity

N = 8192
T = 8
D = 256          # H * IN
OUT = 64
G3 = 3 * OUT     # 192
P = 128
GT = 3           # node tiles (of 128) per group; group = 384 nodes
F32 = mybir.dt.float32
I32 = mybir.dt.int32

_CACHE = {}
_last_in_maps = None


def _build(n_groups):
    """Build + compile the SPMD program for n_groups groups of GT*P nodes."""
    nc = bacc.Bacc("TRN2", target_bir_lowering=False, debug=False, num_devices=T)

    agg = nc.dram_tensor("agg", [N + 1, D], F32, kind="ExternalInput").ap()
    hfull = nc.dram_tensor("hfull", [N + 1, OUT], F32, kind="ExternalInput").ap()
    idx = nc.dram_tensor("idx", [n_groups * GT * P], I32, kind="ExternalInput").ap()
    watt = nc.dram_tensor("watt", [D, OUT], F32, kind="ExternalInput").ap()
    batt = nc.dram_tensor("batt", [OUT], F32, kind="ExternalInput").ap()
    wina = nc.dram_tensor("wina", [OUT + 1, G3], F32, kind="ExternalInput").ap()
    whida = nc.dram_tensor("whida", [OUT + 1, G3], F32, kind="ExternalInput").ap()
    out = nc.dram_tensor("out", [N + 1, OUT], F32, kind="ExternalOutput").ap()

    GP = GT * P       # nodes per group (384)
    GC = GT * 256     # psum_g columns per group (768)

    with tile.TileContext(nc) as tc, ExitStack() as ctx:
        const = ctx.enter_context(tc.tile_pool(name="const", bufs=1))
        sb = ctx.enter_context(tc.tile_pool(name="sb", bufs=2))
        ps1 = ctx.enter_context(tc.tile_pool(name="ps1", bufs=1, space="PSUM"))
        ps2 = ctx.enter_context(tc.tile_pool(name="ps2", bufs=2, space="PSUM"))

        identity = const.tile([P, P], F32)
        make_identity(nc, identity[:])
        watt_sb = const.tile([P, 2 * OUT], F32)
        nc.sync.dma_start(watt_sb[:, 0:OUT], watt[0:P, :])
        nc.sync.dma_start(watt_sb[:, OUT : 2 * OUT], watt[P : 2 * P, :])
        batt_sb = const.tile([OUT, 1], F32)
        nc.sync.dma_start(batt_sb[:], batt[:, None])
        wina_sb = const.tile([OUT + 1, G3], F32)
        nc.sync.dma_start(wina_sb[:], wina[:])
        whida_sb = const.tile([OUT + 1, G3], F32)
        nc.sync.dma_start(whida_sb[:], whida[:])

        for g in range(n_groups):
            base = g * GP

            idx_g = sb.tile([P, GT], I32)
            for j in range(GT):
                nc.sync.dma_start(
                    idx_g[:, j : j + 1], idx[base + j * P : base + (j + 1) * P, None]
                )

            # Gather this group's node rows (agg + h), transposing each x tile
            # onto feature-major layout as soon as it lands.  Relu is fused
            # into the PSUM->SBUF copies below.
            # psum_xT columns: k-half k at [k*GP, (k+1)*GP), tile j at j*P within.
            h_grp = sb.tile([P, GT * (OUT + 1)], F32)
            hv = h_grp[:].rearrange("p (t c) -> p t c", c=OUT + 1)
            psum_xT = ps1.tile([P, 2 * GP], F32, space="PSUM")
            psum_hT = ps1.tile([OUT + 1, GP], F32, space="PSUM")
            nc.gpsimd.memset(hv[:, :, OUT : OUT + 1], 1.0)
            for j in range(GT):
                x_sb = sb.tile([P, D], F32, tag="x_sb")
                nc.gpsimd.indirect_dma_start(
                    out=x_sb[:],
                    out_offset=None,
                    in_=agg[:],
                    in_offset=IndirectOffsetOnAxis(ap=idx_g[:, j : j + 1], axis=0),
                )
                nc.gpsimd.indirect_dma_start(
                    out=h_grp[:, j * (OUT + 1) : j * (OUT + 1) + OUT],
                    out_offset=None,
                    in_=hfull[:],
                    in_offset=IndirectOffsetOnAxis(ap=idx_g[:, j : j + 1], axis=0),
                )
                for k in range(2):
                    nc.tensor.transpose(
                        out=psum_xT[:, k * GP + j * P : k * GP + (j + 1) * P],
                        in_=x_sb[:, k * P : (k + 1) * P],
                        identity=identity[:],
                    )
                nc.tensor.transpose(
                    out=psum_hT[:, j * P : (j + 1) * P],
                    in_=h_grp[:, j * (OUT + 1) : (j + 1) * (OUT + 1)],
                    identity=identity[:],
                )
            xT = sb.tile([P, 2 * GP], F32)
            nc.vector.tensor_scalar_max(xT[:, 0:GP], psum_xT[:, 0:GP], 0.0)
            nc.scalar.activation(
                xT[:, GP : 2 * GP],
                psum_xT[:, GP : 2 * GP],
                mybir.ActivationFunctionType.Relu,
            )
            hT = sb.tile([OUT + 1, GP], F32)
            nc.vector.tensor_copy(hT[:], psum_hT[:])

            # Phase A: yT = relu(W_att^T @ relu(x)^T + b_att)   [64, GP]
            psum_A = ps2.tile([OUT, GP], F32, space="PSUM")
            nc.tensor.matmul(
                psum_A[:], watt_sb[:, 0:OUT], xT[:, 0:GP], start=True, stop=False
            )
            nc.tensor.matmul(
                psum_A[:],
                watt_sb[:, OUT : 2 * OUT],
                xT[:, GP : 2 * GP],
                start=False,
                stop=True,
            )
            yT = sb.tile([OUT + 1, GP], F32)
            nc.scalar.activation(
                yT[0:OUT, :],
                psum_A[:],
                mybir.ActivationFunctionType.Relu,
                bias=batt_sb[:],
            )
            nc.gpsimd.memset(yT[OUT : OUT + 1, :], 1.0)

            # GRU matmuls, node-major.  One PSUM bank (512 cols) per node tile
            # (start=True claims a whole 2KB zero region, so accumulation
            # groups must not share a bank).  Within tile j's bank: columns
            # [0, 192) hold gi (+gh accumulated for r,z); [192, 256) hold gh_n.
            psum_g = ps1.tile([P, GT * 512], F32, space="PSUM")
            for j in range(GT):
                c0 = j * 512
                ysl = yT[:, j * P : (j + 1) * P]
                hsl = hT[:, j * P : (j + 1) * P]
                nc.tensor.matmul(
                    psum_g[:, c0 : c0 + 192], ysl, wina_sb[:],
                    start=True, stop=False,
                )
                nc.tensor.matmul(
                    psum_g[:, c0 : c0 + 128], hsl, whida_sb[:, 0:128],
                    start=False, stop=False,
                )
                nc.tensor.matmul(
                    psum_g[:, c0 + 192 : c0 + 256], hsl, whida_sb[:, 128:192],
                    start=False, stop=True,
                )

            # Gates, batched over the group via strided views.
            gv = psum_g[:].rearrange("p (t c) -> p t c", c=512)
            r_sb = sb.tile([P, GT * OUT], F32)
            rv = r_sb[:].rearrange("p (t c) -> p t c", c=OUT)
            nc.scalar.activation(rv, gv[:, :, 0:OUT], mybir.ActivationFunctionType.Sigmoid)
            z_sb = sb.tile([P, GT * OUT], F32)
            zv = z_sb[:].rearrange("p (t c) -> p t c", c=OUT)
            nc.scalar.activation(
                zv, gv[:, :, OUT : 2 * OUT], mybir.ActivationFunctionType.Sigmoid
            )
            t_sb = sb.tile([P, GT * OUT], F32)
            tv = t_sb[:].rearrange("p (t c) -> p t c", c=OUT)
            nc.vector.tensor_mul(tv, rv, gv[:, :, 3 * OUT : 4 * OUT])
            npre = sb.tile([P, GT * OUT], F32)
            npv = npre[:].rearrange("p (t c) -> p t c", c=OUT)
            nc.vector.tensor_add(npv, tv, gv[:, :, 2 * OUT : 3 * OUT])
            n_sb = sb.tile([P, GT * OUT], F32)
            nv = n_sb[:].rearrange("p (t c) -> p t c", c=OUT)
            nc.scalar.activation(nv, npv, mybir.ActivationFunctionType.Tanh)
            # h_new = n + z * (h - n)
            d_sb = sb.tile([P, GT * OUT], F32)
            dv = d_sb[:].rearrange("p (t c) -> p t c", c=OUT)
            nc.vector.tensor_sub(dv, hv[:, :, 0:OUT], nv)
            t2 = sb.tile([P, GT * OUT], F32)
            t2v = t2[:].rearrange("p (t c) -> p t c", c=OUT)
            nc.vector.tensor_mul(t2v, zv, dv)
            hnew = sb.tile([P, GT * OUT], F32)
            hnv = hnew[:].rearrange("p (t c) -> p t c", c=OUT)
            nc.vector.tensor_add(hnv, nv, t2v)

            for j in range(GT):
                nc.gpsimd.indirect_dma_start(
                    out=out[:],
                    out_offset=IndirectOffsetOnAxis(ap=idx_g[:, j : j + 1], axis=0),
                    in_=hnew[:, j * OUT : (j + 1) * OUT],
                    in_offset=None,
                )

    nc.compile()
    return nc


def kernel(**inputs):
    agg_msg = np.asarray(inputs["agg_msg"], dtype=np.float32)
    h = np.asarray(inputs["h"], dtype=np.float32)
    node_type = np.asarray(inputs["node_type"]).astype(np.int64)
    W_att = np.asarray(inputs["W_att"], dtype=np.float32)
    b_att = np.asarray(inputs["b_att"], dtype=np.float32)
    W_in = np.asarray(inputs["W_in"], dtype=np.float32)
    W_hid = np.asarray(inputs["W_hid"], dtype=np.float32)
    b_in = np.asarray(inputs["b_in"], dtype=np.float32)
    b_hid = np.asarray(inputs["b_hid"], dtype=np.float32)

    idx_lists = [np.where(node_type == t)[0].astype(np.int32) for t in range(T)]
    max_count = max(len(ix) for ix in idx_lists)
    n_groups = -(-max_count // (GT * P))
    k_pad = n_groups * GT * P

    if n_groups not in _CACHE:
        _CACHE[n_groups] = _build(n_groups)
    nc = _CACHE[n_groups]

    agg_pad = np.concatenate([agg_msg, np.zeros((1, D), np.float32)], axis=0)
    h_pad = np.concatenate([h, np.zeros((1, OUT), np.float32)], axis=0)

    in_maps = []
    for t in range(T):
        ix = idx_lists[t]
        ix_pad = np.full(k_pad, N, dtype=np.int32)
        ix_pad[: len(ix)] = ix
        in_maps.append(
            {
                "agg": agg_pad,
                "hfull": h_pad,
                "idx": ix_pad,
                "watt": np.ascontiguousarray(W_att[t]),
                "batt": np.ascontiguousarray(b_att[t]),
                "wina": np.ascontiguousarray(
                    np.concatenate([W_in[t], b_in[t][None, :]], axis=0)
                ),
                "whida": np.ascontiguousarray(
                    np.concatenate([W_hid[t], b_hid[t][None, :]], axis=0)
                ),
            }
        )

    global _last_in_maps
    _last_in_maps = in_maps
    res = run_bass_kernel_spmd(nc, in_maps, core_ids=list(range(T)))

    out_full = np.zeros((N, OUT), dtype=np.float32)
    for t in range(T):
        ix = idx_lists[t]
        out_full[ix] = res.results[t]["out"][ix]
    return out_full


# revision 9
# speedup vs baseline: 1.4954x; 1.4954x over previous
"""Trainium2 Bass kernel for nn_Aggregation_Module (GNN per-node-type GRU update).

Computation (per node n, with t = node_type[n]):
    x  = relu(agg_msg[n])                  # [256]
    y  = relu(x @ W_att[t] + b_att[t])     # [64]
    gi = y @ W_in[t] + b_in[t]             # [192]
    gh = h[n] @ W_hid[t] + b_hid[t]        # [192]
    r  = sigmoid(gi_r + gh_r); z = sigmoid(gi_z + gh_z)
    n_ = tanh(gi_n + r * gh_n)
    h_new = (1 - z) * n_ + z * h[n]

Sharding: type-parallel across the 8 cores (T == 8).  Core t processes all
nodes of type t (counts ~1024 each), so each core needs only its own type's
small weight set (no weight replication).  Node rows are gathered on device
with dma_gather using a per-core row-index list (built on host from
node_type) over a host-concatenated [agg | h] array, and results are
scattered back to their original rows on device with dma_scatter_add into a
zero-initialized output.  Row N (8192) of the padded input/output is a
scratch row targeted by the padding lanes, which keeps the valid-index count
identical on every core (SPMD requires one shared program).
"""

import sys

sys.path.insert(0, "/opt/trn_rl_repo")

from contextlib import ExitStack

import numpy as np

import concourse.bacc as bacc
import concourse.mybir as mybir
import concourse.tile as tile
from concourse.bass_utils import run_bass_kernel_spmd
from concourse.masks import make_identity

N = 8192
T = 8
D = 256          # H * IN
OUT = 64
XH = D + OUT     # 320: concatenated [agg | h] row
P = 128
GT = 3           # node tiles (of 128) per group; group = 384 nodes
F32 = mybir.dt.float32
I16 = mybir.dt.int16

_CACHE = {}
_last_in_maps = None


def _build(n_groups):
    """Build + compile the SPMD program for n_groups groups of GT*P nodes."""
    nc = bacc.Bacc("TRN2", target_bir_lowering=False, debug=False, num_devices=T)

    nt = n_groups * GT            # node tiles of 128
    k_pad = nt * P                # padded per-core node count
    GP = GT * P                   # nodes per group (384)

    xh = nc.dram_tensor("xh", [N + 1, XH], F32, kind="ExternalInput").ap()
    idx = nc.dram_tensor("idx", [P, k_pad // 16], I16, kind="ExternalInput").ap()
    watt = nc.dram_tensor("watt", [D, OUT], F32, kind="ExternalInput").ap()
    batt = nc.dram_tensor("batt", [OUT], F32, kind="ExternalInput").ap()
    # [W_in_rzn (0:192) | zeros (192:256)] so the gi matmul covers the same
    # 256-col range as the gh matmul (uniform overwrite-then-accumulate).
    wina = nc.dram_tensor("wina", [OUT + 1, 4 * OUT], F32, kind="ExternalInput").ap()
    # [W_hid_rz (0:128) | zeros (128:192) | W_hid_n (192:256)] so one matmul
    # accumulates gh_rz onto gi and writes gh_n, leaving gi_n untouched.
    whida = nc.dram_tensor("whida", [OUT + 1, 4 * OUT], F32, kind="ExternalInput").ap()
    out = nc.dram_tensor("out", [N + 1, OUT], F32, kind="ExternalOutput").ap()

    with tile.TileContext(nc) as tc, ExitStack() as ctx:
        const = ctx.enter_context(tc.tile_pool(name="const", bufs=1))
        sb = ctx.enter_context(tc.tile_pool(name="sb", bufs=2))
        ps1 = ctx.enter_context(tc.tile_pool(name="ps1", bufs=1, space="PSUM"))
        ps2 = ctx.enter_context(tc.tile_pool(name="ps2", bufs=2, space="PSUM"))

        identity = const.tile([P, P], F32)
        make_identity(nc, identity[:])
        idx_sb = const.tile([P, k_pad // 16], I16)
        nc.sync.dma_start(idx_sb[:], idx[:])
        watt_sb = const.tile([P, 2 * OUT], F32)
        nc.sync.dma_start(watt_sb[:, 0:OUT], watt[0:P, :])
        nc.sync.dma_start(watt_sb[:, OUT : 2 * OUT], watt[P : 2 * P, :])
        batt_sb = const.tile([OUT, 1], F32)
        nc.sync.dma_start(batt_sb[:], batt[:, None])
        wina_sb = const.tile([OUT + 1, 4 * OUT], F32)
        nc.sync.dma_start(wina_sb[:], wina[:])
        whida_sb = const.tile([OUT + 1, 4 * OUT], F32)
        nc.sync.dma_start(whida_sb[:], whida[:])

        # Whole-core gathered [agg | h] rows and computed h_new rows.
        xh_all = const.tile([P, nt * XH], F32)
        xhv = xh_all[:].rearrange("p (t c) -> p t c", c=XH)
        hnew_all = const.tile([P, nt * OUT], F32)
        hnv_all = hnew_all[:].rearrange("p (t c) -> p t c", c=OUT)

        for g in range(n_groups):
            t0 = g * GT
            nc.gpsimd.dma_gather(
                out_ap=xhv[:, t0 : t0 + GT, :],
                in_ap=xh[:],
                idxs_ap=idx_sb[:, g * (GP // 16) : (g + 1) * (GP // 16)],
                num_idxs=GP,
                num_idxs_reg=GP,
                elem_size=XH,
            )

            # Transpose x chunks and h of each tile onto feature-major layout.
            # psum_xT columns: k-half k at [k*GP, (k+1)*GP), tile j at j*P.
            psum_xT = ps1.tile([P, 2 * GP], F32, space="PSUM")
            psum_hT = ps1.tile([OUT, GP], F32, space="PSUM")
            for j in range(GT):
                for k in range(2):
                    nc.tensor.transpose(
                        out=psum_xT[:, k * GP + j * P : k * GP + (j + 1) * P],
                        in_=xhv[:, t0 + j, k * P : (k + 1) * P],
                        identity=identity[:],
                    )
                nc.tensor.transpose(
                    out=psum_hT[:, j * P : (j + 1) * P],
                    in_=xhv[:, t0 + j, D:XH],
                    identity=identity[:],
                )
            # Relu is fused into the PSUM->SBUF copies (split DVE/ACT).
            xT = sb.tile([P, 2 * GP], F32)
            nc.vector.tensor_scalar_max(xT[:, 0:GP], psum_xT[:, 0:GP], 0.0)
            nc.scalar.activation(
                xT[:, GP : 2 * GP],
                psum_xT[:, GP : 2 * GP],
                mybir.ActivationFunctionType.Relu,
            )
            hT = sb.tile([OUT + 1, GP], F32)
            nc.vector.tensor_copy(hT[0:OUT, :], psum_hT[:])
            nc.gpsimd.memset(hT[OUT : OUT + 1, :], 1.0)

            # Phase A: yT = relu(W_att^T @ relu(x)^T + b_att)   [64, GP]
            psum_A = ps2.tile([OUT, GP], F32, space="PSUM")
            nc.tensor.matmul(
                psum_A[:], watt_sb[:, 0:OUT], xT[:, 0:GP], start=True, stop=False
            )
            nc.tensor.matmul(
                psum_A[:],
                watt_sb[:, OUT : 2 * OUT],
                xT[:, GP : 2 * GP],
                start=False,
                stop=True,
            )
            yT = sb.tile([OUT + 1, GP], F32)
            nc.scalar.activation(
                yT[0:OUT, :],
                psum_A[:],
                mybir.ActivationFunctionType.Relu,
                bias=batt_sb[:],
            )
            nc.gpsimd.memset(yT[OUT : OUT + 1, :], 1.0)

            # GRU matmuls, node-major.  Tile j owns psum_g cols [256j, 256j+256):
            # [0:192) = gi (then += gh_rz on [0:128)); [192:256) = gh_n.
            # Two tiles share a 512-col bank; groups are opened and closed
            # strictly per tile so only one accumulation group is pending per
            # bank at any time.
            psum_g = ps1.tile([P, GT * 256], F32, space="PSUM")
            for j in range(GT):
                c0 = j * 256
                nc.tensor.matmul(
                    psum_g[:, c0 : c0 + 256],
                    yT[:, j * P : (j + 1) * P],
                    wina_sb[:],
                    start=True,
                    stop=False,
                )
                nc.tensor.matmul(
                    psum_g[:, c0 : c0 + 256],
                    hT[:, j * P : (j + 1) * P],
                    whida_sb[:],
                    start=False,
                    stop=True,
                )

            # Gates, batched over the group via strided views.
            gv = psum_g[:].rearrange("p (t c) -> p t c", c=256)
            r_sb = sb.tile([P, GT * OUT], F32)
            rv = r_sb[:].rearrange("p (t c) -> p t c", c=OUT)
            nc.scalar.activation(rv, gv[:, :, 0:OUT], mybir.ActivationFunctionType.Sigmoid)
            z_sb = sb.tile([P, GT * OUT], F32)
            zv = z_sb[:].rearrange("p (t c) -> p t c", c=OUT)
            nc.scalar.activation(
                zv, gv[:, :, OUT : 2 * OUT], mybir.ActivationFunctionType.Sigmoid
            )
            t_sb = sb.tile([P, GT * OUT], F32)
            tv = t_sb[:].rearrange("p (t c) -> p t c", c=OUT)
            nc.vector.tensor_mul(tv, rv, gv[:, :, 3 * OUT : 4 * OUT])
            npre = sb.tile([P, GT * OUT], F32)
            npv = npre[:].rearrange("p (t c) -> p t c", c=OUT)
            nc.vector.tensor_add(npv, tv, gv[:, :, 2 * OUT : 3 * OUT])
            n_sb = sb.tile([P, GT * OUT], F32)
            nv = n_sb[:].rearrange("p (t c) -> p t c", c=OUT)
            nc.scalar.activation(nv, npv, mybir.ActivationFunctionType.Tanh)
            # h_new = n + z * (h - n)
            d_sb = sb.tile([P, GT * OUT], F32)
            dv = d_sb[:].rearrange("p (t c) -> p t c", c=OUT)
            nc.vector.tensor_sub(dv, xhv[:, t0 : t0 + GT, D:XH], nv)
            t2 = sb.tile([P, GT * OUT], F32)
            t2v = t2[:].rearrange("p (t c) -> p t c", c=OUT)
            nc.vector.tensor_mul(t2v, zv, dv)
            nc.vector.tensor_add(hnv_all[:, t0 : t0 + GT, :], nv, t2v)

            nc.gpsimd.dma_scatter_add(
                out_ap=out[:],
                in_ap=hnv_all[:, t0 : t0 + GT, :],
                idxs_ap=idx_sb[:, g * (GP // 16) : (g + 1) * (GP // 16)],
                num_idxs=GP,
                num_idxs_reg=GP,
                elem_size=OUT,
            )

    nc.compile()
    return nc


def kernel(**inputs):
    agg_msg = np.asarray(inputs["agg_msg"], dtype=np.float32)
    h = np.asarray(inputs["h"], dtype=np.float32)
    node_type = np.asarray(inputs["node_type"]).astype(np.int64)
    W_att = np.asarray(inputs["W_att"], dtype=np.float32)
    b_att = np.asarray(inputs["b_att"], dtype=np.float32)
    W_in = np.asarray(inputs["W_in"], dtype=np.float32)
    W_hid = np.asarray(inputs["W_hid"], dtype=np.float32)
    b_in = np.asarray(inputs["b_in"], dtype=np.float32)
    b_hid = np.asarray(inputs["b_hid"], dtype=np.float32)

    idx_lists = [np.where(node_type == t)[0].astype(np.int64) for t in range(T)]
    max_count = max(len(ix) for ix in idx_lists)
    n_groups = -(-max_count // (GT * P))
    k_pad = n_groups * GT * P

    if n_groups not in _CACHE:
        _CACHE[n_groups] = _build(n_groups)
    nc = _CACHE[n_groups]

    xh_pad = np.zeros((N + 1, XH), np.float32)
    xh_pad[:N, :D] = agg_msg
    xh_pad[:N, D:] = h

    in_maps = []
    for t in range(T):
        ix = idx_lists[t]
        ix_pad = np.full(k_pad, N, dtype=np.int16)
        ix_pad[: len(ix)] = ix.astype(np.int16)
        # Wrapped [16, n/16] layout (index i at partition i%16, col i//16),
        # replicated across all 8 GPSIMD cores' partition groups.
        idx_wrapped = np.tile(ix_pad.reshape(k_pad // 16, 16).T, (8, 1))
        win_pad = np.zeros((OUT + 1, 4 * OUT), np.float32)
        win_pad[:OUT, 0:192] = W_in[t]
        win_pad[OUT, 0:192] = b_in[t]
        whid_pad = np.zeros((OUT + 1, 4 * OUT), np.float32)
        whid_pad[:OUT, 0:128] = W_hid[t][:, 0:128]
        whid_pad[OUT, 0:128] = b_hid[t][0:128]
        whid_pad[:OUT, 192:256] = W_hid[t][:, 128:192]
        whid_pad[OUT, 192:256] = b_hid[t][128:192]
        in_maps.append(
            {
                "xh": xh_pad,
                "idx": idx_wrapped,
                "watt": np.ascontiguousarray(W_att[t]),
                "batt": np.ascontiguousarray(b_att[t]),
                "wina": win_pad,
                "whida": whid_pad,
            }
        )

    global _last_in_maps
    _last_in_maps = in_maps
    res = run_bass_kernel_spmd(nc, in_maps, core_ids=list(range(T)))

    out_full = np.zeros((N, OUT), dtype=np.float32)
    for t in range(T):
        ix = idx_lists[t]
        out_full[ix] = res.results[t]["out"][ix]
    return out_full
